# revision 1
# baseline (speedup 1.0000x reference)
"""Trainium2 Bass kernel for nn_CriticalityDistillation.

Computation (see reference): for states [L,B,T,D]
  fe[l,b,t,d] = mean of states^2 over window [t+1, t+1+H) (clipped to T)
  event mask  = top-k of flat pressure (k = round(0.05*B*T))
  obs         = mean fe over non-event positions        -> new_baseline (EMA)
  evidence    = mean over events of relu(fe - new_baseline)
  score       = age-weighted average over bank_evidence
Output: [3, L, D] = stack(evidence, new_baseline, score).

Strategy: shard over L (1 layer per NeuronCore, 8 cores).  On device,
everything is driven off sq = states^2 in [128-position, D] tiles:
  * sum_all fe  == sum_u w_u * sq[u,:]  (window-mean linearity) -> one
    accumulating matmul chain with a constant host-built weight vector.
  * per-event fe rows -> matmuls with host-built sparse selector matrices
    (events are data-dependent; the program is compiled per input).
  * evidence via relu(x-nb) = max(x,nb)-nb so zero-padded slots vanish.
  * score -> matvec with host-folded age weights.
Big matmuls run in bf16 (full PE rate); score stays fp32.
"""

import numpy as np

EVENT_FRAC = 0.05
DECAY = 0.99
HALF_LIFE = 256.0
N_CORES = 8
SC = 512                    # flat positions per superchunk
PW = 128                    # partitions
NBLK = SC // PW             # 4 column blocks per superchunk tile

LAST_RESULT = None          # BassKernelResults of the most recent run (for test.py)
_PLAN_CACHE = {}


def _host_plan(pressure, bank_step, current_step, horizon_H, B, T, D, TTL):
    """All data-dependent constants the device program needs."""
    H = int(horizon_H)
    cur = int(current_step)
    total = B * T
    k = int(round(EVENT_FRAC * total))
    assert T % SC == 0 and H <= SC
    nsc = total // SC
    sc_per_b = T // SC

    # --- event mask: top-k of flat pressure
    flat = np.ascontiguousarray(pressure, dtype=np.float32).reshape(-1)
    idx = np.argpartition(-flat, k - 1)[:k]
    ev = np.sort(idx)                       # flat positions, ascending
    c_of = np.minimum(H, T - 1 - (ev % T))  # window length per event

    # --- w_u: weight of sq[u] in sum over ALL positions of fe (per b)
    w = np.zeros(T, dtype=np.float64)
    t = np.arange(T)
    c_t = np.minimum(H, T - 1 - t)
    for tt in range(T):
        c = int(c_t[tt])
        if c > 0:
            w[tt + 1:tt + 1 + c] += 1.0 / c
    w = w.astype(np.float32)

    # wcol[p, nblk*j + f] = w at flat position SC*j + NBLK*p + f
    wflat = np.tile(w, B)
    wcol = wflat.reshape(nsc, PW, NBLK).transpose(1, 0, 2).reshape(PW, nsc * NBLK)
    wcol = np.ascontiguousarray(wcol, dtype=np.float32)

    # --- events per superchunk
    groups = []          # per j: (positions array, c array)
    for j in range(nsc):
        sel = (ev >= j * SC) & (ev < (j + 1) * SC)
        groups.append((ev[sel], c_of[sel]))
    n = [len(g[0]) for g in groups]
    n_real = int(sum(n))
    assert n_real == k

    def a32(v):
        return (v + 31) & ~31

    # 32-aligned global slot offsets (compute-engine SBUF partition-base rule)
    slot0 = np.zeros(nsc + 1, dtype=int)
    for j in range(nsc):
        slot0[j + 1] = slot0[j] + a32(n[j])
    n_slots = int(slot0[-1])
    n_blocks_fe = max((n_slots + PW - 1) // PW, 1)
    n_slots = n_blocks_fe * PW

    # --- selector matrices per (j, f): [PW, M_j]
    # cols [0:n_j) = own events; cols [a32(n_j) : a32(n_j)+n_{j-1}) = prev tail
    M, tail0 = [], []
    col_off = np.zeros((nsc, NBLK), dtype=int)
    smat_cols = 0
    for j in range(nsc):
        prev = n[j - 1] if (j % sc_per_b != 0) else 0
        t0 = a32(n[j]) if prev > 0 else n[j]
        tail0.append(t0)
        M.append(t0 + prev)
        assert M[j] <= PW, f"event-group overflow M[{j}]={M[j]}"
        for f in range(NBLK):
            col_off[j, f] = smat_cols
            smat_cols += M[j]
    smat = np.zeros((PW, max(smat_cols, 1)), dtype=np.float32)
    for j in range(nsc):
        cols = [(groups[j][0][i], groups[j][1][i], i) for i in range(n[j])]
        if j % sc_per_b != 0 and n[j - 1] > 0:
            cols += [(groups[j - 1][0][i], groups[j - 1][1][i], tail0[j] + i)
                     for i in range(n[j - 1])]
        for f in range(NBLK):
            base = col_off[j, f]
            for (fe_pos, c, ci) in cols:
                if c <= 0:
                    continue
                # rows p with fe_pos+1 <= SC*j + NBLK*p + f <= fe_pos+c
                lo = -(-(int(fe_pos) + 1 - SC * j - f) // NBLK)   # ceil div
                hi = (int(fe_pos) + int(c) - SC * j - f) // NBLK
                lo, hi = max(lo, 0), min(hi, PW - 1)
                if lo <= hi:
                    smat[lo:hi + 1, base + ci] = np.float32(1.0 / c)

    # --- fe_ev destination segments per group: (blk, part, grp_off, cnt)
    # pieces of <=32 rows so every SBUF/PSUM partition base stays 32-aligned
    segs = []
    for j in range(nsc):
        s = []
        g0, cnt = int(slot0[j]), n[j]
        done = 0
        while done < cnt:
            sl = g0 + done
            blk, part = sl // PW, sl % PW
            m = min(32, cnt - done)
            s.append((blk, part, done, m))
            done += m
        segs.append(s)

    # --- bank weights folded with normalization (per layer)
    bs = np.asarray(bank_step)
    valid = (bs >= 0).astype(np.float32)
    age = np.clip(cur - bs, 0, None).astype(np.float32)
    weight = np.exp2(-age / np.float32(HALF_LIFE)) * valid
    ws = weight.sum(axis=1, keepdims=True)
    scale = np.where(ws > 0, 1.0 / np.maximum(ws, 1e-12), 0.0).astype(np.float32)
    wbank = (weight * scale).astype(np.float32)          # [L, TTL]
    nbk = TTL // (2 * PW)                                 # bank tiles per layer
    # wbcol[l][p, 2c+g] = wbank[l, 256c + 2p + g]
    wbcol = wbank.reshape(-1, nbk, PW, 2).transpose(0, 2, 1, 3).reshape(-1, PW, nbk * 2)
    wbcol = np.ascontiguousarray(np.swapaxes(wbcol, 1, 1))

    return dict(H=H, k=k, total=total, nsc=nsc, sc_per_b=sc_per_b, n=n,
                M=M, tail0=tail0, slot0=slot0, n_real=n_real,
                n_blocks_fe=n_blocks_fe, n_slots=n_slots,
                smat=smat, smat_cols=smat_cols, col_off=col_off, segs=segs,
                wcol=wcol, wbcol=wbcol, nbk=nbk, D=D, TTL=TTL)


def _build_program(plan):
    """Build the SPMD Bass/Tile program (one layer per core)."""
    from contextlib import ExitStack
    import concourse.bass as bass
    import concourse.tile as tile
    from concourse import bacc, mybir

    f32 = mybir.dt.float32
    bf16 = mybir.dt.bfloat16
    D = plan['D']
    nsc, sc_per_b = plan['nsc'], plan['sc_per_b']
    n, M, segs, col_off = plan['n'], plan['M'], plan['segs'], plan['col_off']
    tail0 = plan['tail0']
    nbk = plan['nbk']
    nfe = plan['n_blocks_fe']
    smat_cols = plan['smat_cols']
    c_obs = float((1.0 - DECAY) / (plan['total'] - plan['k']))
    inv_k = 1.0 / plan['k']
    n_slots = plan['n_slots']

    nc = bacc.Bacc("TRN2", target_bir_lowering=False, debug=False,
                   num_devices=N_CORES)
    x_d = nc.dram_tensor("x", [nsc, PW, NBLK * D], f32, kind="ExternalInput").ap()
    bank_d = nc.dram_tensor("bank", [nbk, PW, 2 * D], f32, kind="ExternalInput").ap()
    bsc_d = nc.dram_tensor("bsc", [1, D], f32, kind="ExternalInput").ap()
    wcol_d = nc.dram_tensor("wcol", [PW, nsc * NBLK], bf16, kind="ExternalInput").ap()
    wbcol_d = nc.dram_tensor("wbcol", [PW, nbk * 2], f32, kind="ExternalInput").ap()
    smat_d = nc.dram_tensor("smat", [PW, smat_cols], bf16, kind="ExternalInput").ap()
    out_d = nc.dram_tensor("out", [3, D], f32, kind="ExternalOutput").ap()

    with tile.TileContext(nc) as tc, ExitStack() as ctx:
        p_const = ctx.enter_context(tc.tile_pool(name="const", bufs=1))
        p_x = ctx.enter_context(tc.tile_pool(name="x", bufs=3))
        p_sq = ctx.enter_context(tc.tile_pool(name="sq", bufs=4))
        p_bk = ctx.enter_context(tc.tile_pool(name="bk", bufs=2))
        p_small = ctx.enter_context(tc.tile_pool(name="small", bufs=1))
        ps_tot = ctx.enter_context(tc.tile_pool(name="ptot", bufs=1, space="PSUM"))
        ps_ev = ctx.enter_context(tc.tile_pool(name="pev", bufs=2, space="PSUM"))
        ps_sc = ctx.enter_context(tc.tile_pool(name="psc", bufs=1, space="PSUM"))

        # constants
        smat_sb = p_const.tile([PW, smat_cols], bf16)
        nc.sync.dma_start(out=smat_sb, in_=smat_d)
        wcol_sb = p_const.tile([PW, nsc * NBLK], bf16)
        nc.sync.dma_start(out=wcol_sb, in_=wcol_d)
        wbcol_sb = p_const.tile([PW, nbk * 2], f32)
        nc.sync.dma_start(out=wbcol_sb, in_=wbcol_d)
        bsc_sb = p_const.tile([1, D], f32)
        nc.sync.dma_start(out=bsc_sb, in_=bsc_d)
        ones_sb = p_const.tile([PW, 1], f32)
        nc.vector.memset(ones_sb, 1.0)
        negones_sb = p_const.tile([PW, 1], bf16)
        nc.vector.memset(negones_sb, -1.0)
        fe_ev = p_const.tile([PW, nfe * D], f32)
        nc.gpsimd.memset(fe_ev, 0.0)

        psum_tot = ps_tot.tile([1, D], f32, tag="tot")
        psum_score = ps_sc.tile([1, D], f32)

        bank_js = {3: 0, 6: 1, 9: 2, 12: 3} if nsc == 16 else {
            max(0, (i * nsc) // nbk + 1): i for i in range(nbk)}

        for j in range(nsc):
            x_t = p_x.tile([PW, NBLK * D], f32)
            nc.sync.dma_start(out=x_t, in_=x_d[j])
            sq_t = p_sq.tile([PW, NBLK * D], bf16)
            if j % 2 == 0:
                nc.scalar.activation(out=sq_t, in_=x_t,
                                     func=mybir.ActivationFunctionType.Square)
            else:
                nc.vector.tensor_mul(sq_t, x_t, x_t)

            psum_ev = None
            if M[j] > 0:
                psum_ev = ps_ev.tile([PW, D], f32, tag="ev", name=f"pev{j}")
            for f in range(NBLK):
                for h in range(2):
                    rhs = sq_t[:, f * D + h * 512: f * D + (h + 1) * 512]
                    cidx = NBLK * j + f
                    nc.tensor.matmul(
                        psum_tot[0:1, h * 512:(h + 1) * 512],
                        wcol_sb[:, cidx:cidx + 1], rhs,
                        start=(j == 0 and f == 0), stop=False)
                    if psum_ev is not None:
                        co = int(col_off[j, f])
                        nc.tensor.matmul(
                            psum_ev[0:M[j], h * 512:(h + 1) * 512],
                            smat_sb[:, co:co + M[j]], rhs,
                            start=(f == 0), stop=(f == NBLK - 1))

            # group completions (DVE may read at most ONE PSUM operand):
            # copy own partial -> fe_ev now; next superchunk adds the tail
            # in place (fe_ev += psum tail rows).
            if n[j] > 0:
                for (blk, part, goff, cnt) in segs[j]:
                    dst = fe_ev[part:part + cnt, blk * D:(blk + 1) * D]
                    nc.scalar.copy(dst, psum_ev[goff:goff + cnt, 0:D])
            if j % sc_per_b != 0 and n[j - 1] > 0:
                for (blk, part, goff, cnt) in segs[j - 1]:
                    dst = fe_ev[part:part + cnt, blk * D:(blk + 1) * D]
                    b = psum_ev[tail0[j] + goff:tail0[j] + goff + cnt, 0:D]
                    nc.vector.tensor_add(dst, dst, b)

            # interleave score stream
            if j in bank_js:
                c = bank_js[j]
                bk_t = p_bk.tile([PW, 2 * D], f32)
                nc.sync.dma_start(out=bk_t, in_=bank_d[c])
                for g in range(2):
                    for h in range(2):
                        rhs = bk_t[:, g * D + h * 512: g * D + (h + 1) * 512]
                        widx = 2 * c + g
                        nc.tensor.matmul(
                            psum_score[0:1, h * 512:(h + 1) * 512],
                            wbcol_sb[:, widx:widx + 1], rhs,
                            start=(c == 0 and g == 0),
                            stop=(c == nbk - 1 and g == 1))

        # ---- endgame ----
        # S_all - S_ev: subtract event-fe sums from psum_tot via -1 weights.
        # bf16 shadow keeps the PE at full rate; S_ev only feeds nb at 1e-2
        # weight so bf16 rounding there is negligible.
        fe_bf = p_small.tile([PW, nfe * D], bf16)
        nc.scalar.copy(fe_bf, fe_ev)
        for blk in range(nfe):
            for h in range(2):
                nc.tensor.matmul(
                    psum_tot[0:1, h * 512:(h + 1) * 512],
                    negones_sb,
                    fe_bf[:, blk * D + h * 512: blk * D + (h + 1) * 512],
                    start=False, stop=(blk == nfe - 1))

        nb_sb = p_small.tile([1, D], f32)
        nb = nb_sb[0:1, :]
        nc.vector.tensor_scalar_mul(nb, psum_tot[0:1, :], c_obs)
        nc.vector.tensor_add(nb, nb, bsc_sb)

        nb_b = p_small.tile([PW, D], f32)
        nc.gpsimd.partition_broadcast(nb_b, nb)

        mx = p_small.tile([PW, nfe * D], f32)
        for blk in range(nfe):
            nc.vector.tensor_max(mx[:, blk * D:(blk + 1) * D],
                                 fe_ev[:, blk * D:(blk + 1) * D], nb_b)

        psum_emax = ps_tot.tile([1, D], f32, tag="tot")
        for blk in range(nfe):
            for h in range(2):
                nc.tensor.matmul(
                    psum_emax[0:1, h * 512:(h + 1) * 512],
                    ones_sb,
                    mx[:, blk * D + h * 512: blk * D + (h + 1) * 512],
                    start=(blk == 0), stop=(blk == nfe - 1))

        # evidence = (sum_slots max - n_real*nb - n_pad*max(nb,0)) / k
        t_m0 = p_small.tile([1, D], f32)
        nc.vector.tensor_scalar_max(t_m0, nb, 0.0)
        nc.vector.tensor_scalar_mul(t_m0, t_m0, float((n_slots - plan['n_real']) * inv_k))
        t_e = p_small.tile([1, D], f32)
        nc.vector.tensor_scalar_mul(t_e, psum_emax[0:1, :], inv_k)
        t_1 = p_small.tile([1, D], f32)
        nc.vector.tensor_scalar_mul(t_1, nb, float(plan['n_real'] * inv_k))
        nc.vector.tensor_sub(t_e, t_e, t_1)
        ev_sb = p_small.tile([1, D], f32)
        nc.vector.tensor_sub(ev_sb, t_e, t_m0)

        sc_sb = p_small.tile([1, D], f32)
        nc.scalar.copy(sc_sb, psum_score[0:1, :])
        nc.sync.dma_start(out=out_d[0:1, :], in_=ev_sb)
        nc.sync.dma_start(out=out_d[1:2, :], in_=nb_sb)
        nc.sync.dma_start(out=out_d[2:3, :], in_=sc_sb)

    nc.compile()
    return nc


def _make_in_maps(plan, states, bank_evidence, baseline, L, B, T, D, TTL):
    nsc, nbk = plan['nsc'], plan['nbk']
    import ml_dtypes
    smat = np.ascontiguousarray(plan['smat'].astype(ml_dtypes.bfloat16))
    wcol = np.ascontiguousarray(plan['wcol'].astype(ml_dtypes.bfloat16))
    states = np.ascontiguousarray(states, dtype=np.float32)
    bank = np.ascontiguousarray(bank_evidence, dtype=np.float32)
    baseline = np.asarray(baseline, dtype=np.float32)
    in_maps = []
    for l in range(L):
        in_maps.append({
            "x": states[l].reshape(nsc, PW, NBLK * D),
            "bank": bank[l].reshape(nbk, PW, 2 * D),
            "bsc": (np.float32(DECAY) * baseline[l]).reshape(1, D),
            "wcol": wcol,
            "wbcol": np.ascontiguousarray(plan['wbcol'][l], dtype=np.float32),
            "smat": smat,
        })
    return in_maps


def kernel(pressure, states, bank_evidence, baseline, bank_step,
           current_step, horizon_H):
    global LAST_RESULT
    from concourse.bass_utils import run_bass_kernel_spmd

    states = np.asarray(states)
    L, B, T, D = states.shape
    TTL = np.asarray(bank_evidence).shape[1]
    assert L == N_CORES

    plan = _host_plan(np.asarray(pressure), np.asarray(bank_step),
                      current_step, horizon_H, B, T, D, TTL)

    import hashlib
    hsh = hashlib.sha1()
    hsh.update(plan['smat'].tobytes())
    hsh.update(plan['wcol'].tobytes())
    cache_key = (hsh.hexdigest(), plan['H'], B, T, D, TTL)
    if cache_key in _PLAN_CACHE:
        nc = _PLAN_CACHE[cache_key]
    else:
        nc = _build_program(plan)
        _PLAN_CACHE[cache_key] = nc

    in_maps = _make_in_maps(plan, states, np.asarray(bank_evidence),
                            np.asarray(baseline), L, B, T, D, TTL)
    res = run_bass_kernel_spmd(nc, in_maps, core_ids=list(range(N_CORES)))
    LAST_RESULT = res
    out = np.stack([res.results[l]["out"] for l in range(L)], axis=1)
    return out.astype(np.float32)



# revision 5
# speedup vs baseline: 1.0997x; 1.0997x over previous
"""Trainium2 Bass kernel for nn_CriticalityDistillation.

Computation (see reference): for states [L,B,T,D]
  fe[l,b,t,d] = mean of states^2 over window [t+1, t+1+H) (clipped to T)
  event mask  = top-k of flat pressure (k = round(0.05*B*T))
  obs         = mean fe over non-event positions        -> new_baseline (EMA)
  evidence    = mean over events of relu(fe - new_baseline)
  score       = age-weighted average over bank_evidence
Output: [3, L, D] = stack(evidence, new_baseline, score).

Strategy: shard over L (1 layer per NeuronCore, 8 cores).  On device the
whole event/baseline computation is one matmul stream over sq = states^2:

  * Flat positions are tiled as x[j] = [128 part, 4 sub-pos, D] superchunks
    (512 positions each, 4 superchunks per batch row).  For each (j, f) a
    host-built weight slab W[j,f] in [128, 128] maps position u=512j+4p+f to
    up to 128 "slot" rows of the current block:
      - slot s_tot[j] accumulates w_u * sq[u]  (S_all partial, window-mean
        linearity), and
      - each event e gets a slot row accumulating (1/c) * sq over its
        window [e+1, e+c] -- including the spill into superchunk j from
        events of superchunk j-1 (windows never cross batch rows).
  * One PSUM block [128, D] accumulates a whole batch row (4 superchunks,
    16 matmuls per 512-col half) via per-element has_written semantics:
    start=True only on the block's first matmul.  Zero weight columns pad
    every slab to 128 rows, so all partitions are written and matmul
    output APs are always the full, base-0 partition range.
  * Per block: one [128, D] PSUM->SBUF copy; S_all - S_ev falls out of a
    single +/-1 selector matmul per block; evidence needs only
    max(fe, nb) (DVE) + a +1-selector matmul; score is a small matvec
    stream over the bank, loaded last so it overlaps the endgame.
Event slabs run in bf16 (full PE rate); selector/score matmuls in fp32.
"""

import numpy as np

EVENT_FRAC = 0.05
DECAY = 0.99
HALF_LIFE = 256.0
N_CORES = 8
SC = 512                    # flat positions per superchunk
PW = 128                    # partitions
NBLK = SC // PW             # 4 sub-positions per partition

LAST_RESULT = None          # BassKernelResults of the most recent run (for test.py)
_PROGRAM_CACHE = {}


def _host_plan(pressure, bank_step, current_step, horizon_H, B, T, D, TTL):
    """Host-built weight slabs + selectors (data-dependent; program is not)."""
    H = int(horizon_H)
    cur = int(current_step)
    total = B * T
    k = int(round(EVENT_FRAC * total))
    assert T % SC == 0 and 0 < H <= SC
    nsc = total // SC
    sc_per_b = T // SC
    nbg = nsc // sc_per_b             # blocks (one per batch row)
    assert nsc % sc_per_b == 0

    # --- event mask: top-k of flat pressure
    flat = np.ascontiguousarray(pressure, dtype=np.float32).reshape(-1)
    idx = np.argpartition(-flat, k - 1)[:k]
    ev = np.sort(idx)                        # flat positions, ascending
    c_of = np.minimum(H, T - 1 - (ev % T)).astype(np.int64)

    # --- w_u: weight of sq[u] in sum over ALL positions of fe (per batch row)
    w = np.zeros(T, dtype=np.float64)
    c_t = np.minimum(H, T - 1 - np.arange(T))
    for tt in range(T):
        c = int(c_t[tt])
        if c > 0:
            w[tt + 1:tt + 1 + c] += 1.0 / c
    w_flat = np.tile(w, B)

    # --- block-local slot layout: [tot_j, ev_j0, ...] per superchunk, packed
    ev_j = ev // SC
    n = np.bincount(ev_j, minlength=nsc)
    s_tot = np.zeros(nsc, dtype=int)
    ev_row = np.zeros(max(k, 1), dtype=int)
    ei = 0
    for b in range(nbg):
        s = 0
        for j in range(b * sc_per_b, (b + 1) * sc_per_b):
            s_tot[j] = s
            s += 1
            for _ in range(int(n[j])):
                ev_row[ei] = s
                s += 1
                ei += 1
        assert s <= PW, f"block {b} needs {s} slots > {PW}"

    # --- weight slabs: slab[j, p, f, r] = weight of u=512j+4p+f for slot r
    slab = np.zeros((nsc, PW, NBLK, PW), dtype=np.float64)
    u = np.arange(total)
    np.add.at(slab, (u // SC, (u % SC) // NBLK, u % NBLK, s_tot[u // SC]), w_flat)
    if k:
        us, rows, vals = [], [], []
        for i in range(k):
            e, c, r = int(ev[i]), int(c_of[i]), int(ev_row[i])
            if c <= 0:
                continue
            us.append(np.arange(e + 1, e + c + 1))
            rows.append(np.full(c, r))
            vals.append(np.full(c, 1.0 / c))
        us = np.concatenate(us)
        rows = np.concatenate(rows)
        vals = np.concatenate(vals)
        np.add.at(slab, (us // SC, (us % SC) // NBLK, us % NBLK, rows), vals)
    smat = np.ascontiguousarray(
        slab.transpose(1, 0, 2, 3).reshape(PW, nsc * NBLK * PW), dtype=np.float32)

    # --- block selectors: obs = +1 tot rows, -1 event rows; emax = +1 events
    osel = np.zeros((PW, nbg), dtype=np.float32)
    esel = np.zeros((PW, nbg), dtype=np.float32)
    for j in range(nsc):
        osel[s_tot[j], j // sc_per_b] = 1.0
    for i in range(k):
        b = int(ev_j[i]) // sc_per_b
        osel[ev_row[i], b] = -1.0
        esel[ev_row[i], b] = 1.0

    # --- bank weights folded with normalization (per layer)
    bs = np.asarray(bank_step)
    valid = (bs >= 0).astype(np.float32)
    age = np.clip(cur - bs, 0, None).astype(np.float32)
    weight = np.exp2(-age / np.float32(HALF_LIFE)) * valid
    ws = weight.sum(axis=1, keepdims=True)
    scale = np.where(ws > 0, 1.0 / np.maximum(ws, 1e-12), 0.0).astype(np.float32)
    wbank = (weight * scale).astype(np.float32)          # [L, TTL]
    nbk = TTL // (NBLK * PW)                              # bank tiles per layer
    assert TTL % (NBLK * PW) == 0
    # wb[l][p, 4c+g] = wbank[l, 512c + 4p + g]
    wb = wbank.reshape(-1, nbk, PW, NBLK).transpose(0, 2, 1, 3).reshape(-1, PW, nbk * NBLK)

    return dict(k=k, total=total, nsc=nsc, sc_per_b=sc_per_b, nbg=nbg,
                smat=smat, osel=osel, esel=esel, wb=np.ascontiguousarray(wb),
                nbk=nbk, D=D, TTL=TTL)


def _build_program(B, T, D, TTL):
    """Build the SPMD Bass/Tile program (one layer per core, shape-only)."""
    from contextlib import ExitStack
    import concourse.bass as bass
    import concourse.tile as tile
    from concourse import bacc, mybir

    f32 = mybir.dt.float32
    bf16 = mybir.dt.bfloat16
    total = B * T
    k = int(round(EVENT_FRAC * total))
    nsc = total // SC
    sc_per_b = T // SC
    nbg = nsc // sc_per_b
    nbk = TTL // (NBLK * PW)
    c_obs = float((1.0 - DECAY) / (total - k))
    inv_k = 1.0 / k

    nc = bacc.Bacc("TRN2", target_bir_lowering=False, debug=False,
                   num_devices=N_CORES)
    x_d = nc.dram_tensor("x", [nsc, PW, NBLK * D], f32, kind="ExternalInput").ap()
    bank_d = nc.dram_tensor("bank", [nbk, PW, NBLK * D], f32, kind="ExternalInput").ap()
    bsc_d = nc.dram_tensor("bsc", [1, D], f32, kind="ExternalInput").ap()
    smat_d = nc.dram_tensor("smat", [PW, nsc * NBLK * PW], bf16, kind="ExternalInput").ap()
    wb_d = nc.dram_tensor("wb", [PW, nbk * NBLK], f32, kind="ExternalInput").ap()
    osel_d = nc.dram_tensor("osel", [PW, nbg], f32, kind="ExternalInput").ap()
    esel_d = nc.dram_tensor("esel", [PW, nbg], f32, kind="ExternalInput").ap()
    out_d = nc.dram_tensor("out", [1, 3 * D], f32, kind="ExternalOutput").ap()

    with tile.TileContext(nc) as tc, ExitStack() as ctx:
        p_x = ctx.enter_context(tc.tile_pool(name="x", bufs=3))
        p_sq = ctx.enter_context(tc.tile_pool(name="sq", bufs=3))
        p_fe = ctx.enter_context(tc.tile_pool(name="fe", bufs=nbg))
        p_mx = ctx.enter_context(tc.tile_pool(name="mx", bufs=2))
        p_bk = ctx.enter_context(tc.tile_pool(name="bk", bufs=2))
        p_const = ctx.enter_context(tc.tile_pool(name="const", bufs=1))
        p_small = ctx.enter_context(tc.tile_pool(name="small", bufs=1))
        ps_blk = ctx.enter_context(tc.tile_pool(name="pblk", bufs=2, space="PSUM"))
        ps_obs = ctx.enter_context(tc.tile_pool(name="pobs", bufs=1, space="PSUM"))
        ps_sc = ctx.enter_context(tc.tile_pool(name="psc", bufs=1, space="PSUM"))

        # x[0] DMA first (sync ring); constants go on the scalar HWDGE ring so
        # they share SDMA bandwidth instead of queueing behind the x stream.
        x_tiles = {}

        def load_x(j):
            t = p_x.tile([PW, NBLK * D], f32, name=f"x{j}", tag="x")
            nc.sync.dma_start(out=t, in_=x_d[j])
            x_tiles[j] = t

        load_x(0)
        smat_sb = p_const.tile([PW, nsc * NBLK * PW], bf16)
        nc.scalar.dma_start(out=smat_sb, in_=smat_d)
        wb_sb = p_const.tile([PW, nbk * NBLK], f32)
        nc.scalar.dma_start(out=wb_sb, in_=wb_d)
        osel_sb = p_const.tile([PW, nbg], f32)
        nc.scalar.dma_start(out=osel_sb, in_=osel_d)
        esel_sb = p_const.tile([PW, nbg], f32)
        nc.scalar.dma_start(out=esel_sb, in_=esel_d)
        bsc_sb = p_const.tile([1, D], f32)
        nc.scalar.dma_start(out=bsc_sb, in_=bsc_d)

        obs_t = ps_obs.tile([1, D], f32)
        half = NBLK * D // 2
        fe_tiles = []
        blk_t = None
        for j in range(nsc):
            if j + 1 < nsc:
                load_x(j + 1)
            x_t = x_tiles.pop(j)
            sq_t = p_sq.tile([PW, NBLK * D], bf16, name=f"sq{j}", tag="sq")
            # split the square across both elementwise engines
            nc.scalar.activation(out=sq_t[:, 0:half], in_=x_t[:, 0:half],
                                 func=mybir.ActivationFunctionType.Square)
            nc.vector.tensor_mul(sq_t[:, half:], x_t[:, half:], x_t[:, half:])

            jb = j % sc_per_b
            if jb == 0:
                blk_t = ps_blk.tile([PW, D], f32, name=f"blk{j // sc_per_b}", tag="blk")
            for f in range(NBLK):
                Wc = smat_sb[:, (j * NBLK + f) * PW:(j * NBLK + f + 1) * PW]
                for h in range(2):
                    nc.tensor.matmul(
                        blk_t[0:PW, h * 512:(h + 1) * 512],
                        Wc, sq_t[:, f * D + h * 512: f * D + (h + 1) * 512],
                        start=(jb == 0 and f == 0),
                        stop=(jb == sc_per_b - 1 and f == NBLK - 1))

            if jb == sc_per_b - 1:
                b = j // sc_per_b
                fe_t = p_fe.tile([PW, D], f32, name=f"fe{b}", tag="fe")
                nc.scalar.copy(fe_t, blk_t[0:PW, 0:D])
                fe_tiles.append(fe_t)
                for h in range(2):
                    nc.tensor.matmul(
                        obs_t[0:1, h * 512:(h + 1) * 512],
                        osel_sb[:, b:b + 1], fe_t[:, h * 512:(h + 1) * 512],
                        start=(b == 0), stop=(b == nbg - 1))

        # bank DMAs queue on the sync ring right behind the x stream
        bk_tiles = []
        for c in range(nbk):
            bk_t = p_bk.tile([PW, NBLK * D], f32, name=f"bk{c}", tag="bk")
            nc.sync.dma_start(out=bk_t, in_=bank_d[c])
            bk_tiles.append(bk_t)

        # ---- endgame ----
        out_sb = p_small.tile([1, 3 * D], f32)
        nb = out_sb[0:1, D:2 * D]
        t1 = p_small.tile([1, D], f32)
        nc.vector.tensor_scalar_mul(t1, obs_t[0:1, :], c_obs)
        nc.vector.tensor_add(nb, t1, bsc_sb)

        # broadcast nb to 128 partitions
        nb_b = p_small.tile([PW, D], f32)
        nc.gpsimd.partition_broadcast(nb_b, nb)

        em_t = ps_blk.tile([1, D], f32, name="emax", tag="blk")
        for b in range(nbg):
            mx = p_mx.tile([PW, D], f32, name=f"mx{b}", tag="mx")
            nc.vector.tensor_max(mx, fe_tiles[b], nb_b)
            for h in range(2):
                nc.tensor.matmul(em_t[0:1, h * 512:(h + 1) * 512],
                                 esel_sb[:, b:b + 1], mx[:, h * 512:(h + 1) * 512],
                                 start=(b == 0), stop=(b == nbg - 1))

        t2 = p_small.tile([1, D], f32)
        nc.vector.tensor_scalar_mul(t2, em_t[0:1, :], inv_k)
        nc.vector.tensor_sub(out_sb[0:1, 0:D], t2, nb)

        # score stream (PE work emitted last; overlaps the endgame via deps)
        sc_t = ps_sc.tile([1, D], f32)
        for c in range(nbk):
            for g in range(NBLK):
                widx = NBLK * c + g
                for h in range(2):
                    nc.tensor.matmul(
                        sc_t[0:1, h * 512:(h + 1) * 512],
                        wb_sb[:, widx:widx + 1],
                        bk_tiles[c][:, g * D + h * 512: g * D + (h + 1) * 512],
                        start=(c == 0 and g == 0),
                        stop=(c == nbk - 1 and g == NBLK - 1))
        nc.scalar.copy(out_sb[0:1, 2 * D:3 * D], sc_t[0:1, :])

        nc.sync.dma_start(out=out_d, in_=out_sb)

    nc.compile()
    return nc


def _make_in_maps(plan, states, bank_evidence, baseline, L, B, T, D, TTL):
    nsc, nbk = plan['nsc'], plan['nbk']
    import ml_dtypes
    smat = np.ascontiguousarray(plan['smat'].astype(ml_dtypes.bfloat16))
    states = np.ascontiguousarray(states, dtype=np.float32)
    bank = np.ascontiguousarray(bank_evidence, dtype=np.float32)
    baseline = np.asarray(baseline, dtype=np.float32)
    in_maps = []
    for l in range(L):
        in_maps.append({
            "x": states[l].reshape(nsc, PW, NBLK * D),
            "bank": bank[l].reshape(nbk, PW, NBLK * D),
            "bsc": (np.float32(DECAY) * baseline[l]).reshape(1, D),
            "smat": smat,
            "wb": np.ascontiguousarray(plan['wb'][l], dtype=np.float32),
            "osel": plan['osel'],
            "esel": plan['esel'],
        })
    return in_maps


def kernel(pressure, states, bank_evidence, baseline, bank_step,
           current_step, horizon_H):
    global LAST_RESULT
    from concourse.bass_utils import run_bass_kernel_spmd

    states = np.asarray(states)
    L, B, T, D = states.shape
    TTL = np.asarray(bank_evidence).shape[1]
    assert L == N_CORES

    plan = _host_plan(np.asarray(pressure), np.asarray(bank_step),
                      current_step, horizon_H, B, T, D, TTL)

    cache_key = (B, T, D, TTL)
    if cache_key in _PROGRAM_CACHE:
        nc = _PROGRAM_CACHE[cache_key]
    else:
        nc = _build_program(B, T, D, TTL)
        _PROGRAM_CACHE[cache_key] = nc

    in_maps = _make_in_maps(plan, states, np.asarray(bank_evidence),
                            np.asarray(baseline), L, B, T, D, TTL)
    res = run_bass_kernel_spmd(nc, in_maps, core_ids=list(range(N_CORES)))
    LAST_RESULT = res
    out = np.stack([res.results[l]["out"].reshape(3, D) for l in range(L)], axis=1)
    return out.astype(np.float32)
